# revision 22
# baseline (speedup 1.0000x reference)
"""CTC loss (nn.CTCLoss, blank=0, reduction='mean', zero_infinity=True) for
T=160, B=64, C=6625, S=25 on 8 TRN2 NeuronCores.

Sharding: data-parallel over batch - 8 of the 64 samples per core.

Algorithm: the CTC forward DP is lower-triangular in the extended-target
index s (alpha[s] depends only on alpha[s], alpha[s-1], alpha[s-2]), so it
is solved ROW-BY-ROW over s instead of step-by-step over t.  Row s is one
first-order linear recurrence along t,

    alpha_t[s] = (u_t + alpha_{t-1}[s]) * p_t[s],
    u_t        = alpha_{t-1}[s-1] + m[s] * alpha_{t-1}[s-2],

i.e. ONE hardware prefix-scan (tensor_tensor_scan, op0=add/op1=mult, fp32
state) over the whole time axis, preceded by one scalar_tensor_tensor for
the u row - and u degenerates to the raw alpha[s-1] row (pure AP shift into
the zero pad column) whenever m[s] is structurally zero: even s and s=1,
i.e. 27 of the 51 rows need no u op at all.

T is split in half: a forward chain over t<80 and a mirrored backward
(beta) chain over t>=80.  Because the scan is per-partition independent
and its cost depends only on free-axis length, BOTH chains ride in the
SAME instructions: partitions 0-7 carry the 8 forward samples, partitions
8-15 the backward ones (mask/init/score data selects the direction).  Each
row is further split into two column passes (cols [0,TH) and [TH,80), the
second seeded by the first's final state and trailing it by LAG rows as an
independent dependency chain), which hides the ~95ns/op semaphore+SBUF-ack
latency of the serial row chain and leaves the Vector engine >99% busy:
the whole DP is 102 scans + 48 stts of [16, ~40].  The device returns the
two boundary columns (alpha_79 / beta-hat_80, 53 values per sample); the
host contracts them (a 51-element masked dot per sample) while assembling
the final loss - the same finishing step that already applies the log and
the batch mean.

Probabilities stay in fp32 with NO per-step rescaling: the host subtracts
a per-(b,t) centering constant (mean extended-target score + 0.5) from the
scores before upload, which keeps all alpha/beta magnitudes inside fp32
range (empirically |log alpha| < 70); the host adds the centering log-sums
back when assembling the final loss.
"""

import numpy as np

import concourse.bacc as bacc
import concourse.bass as bass
import concourse.mybir as mybir
import concourse.tile as tile
from concourse.bass_utils import run_bass_kernel_spmd

T = 160
B = 64
C = 6625
S = 25
L = 2 * S + 1  # 51
NCORES = 8
BLOC = B // NCORES  # 8 samples per core
T1 = 80  # forward half; backward covers T-T1 = 80
NP = 2 * BLOC  # 16 partitions: 0-7 forward, 8-15 backward
ROWS = L + 2  # 2 leading zero-pad rows (taps s-1, s-2 of rows 0/1)
COLS = T1 + 1  # leading zero-pad column (alpha_{-1})
NODD = 24  # odd rows s=3,5,...,49 need the u-op
SM = 2 * L  # leading ini|msk columns of the staged input
# s-rows per DMA chunk (chunk 0 also carries ini|msk) and per exp op:
# fine-grained early so the row loop's first iterations aren't starved,
# coarse later where the pipeline is far ahead.
CHUNKS = [4, 8, 12, 27]
EXPS = [2, 2, 8, 12, 27]  # padded/truncated to 51 rows
# NOTE: scalar_tensor_tensor on the Pool/gpsimd engine is rejected by
# neuronx-cc codegen ("Instruction engine check failed (Pool)") - the stts
# must stay on the Vector engine.
STT_POOL = 0  # 0: stts on DVE; 1: pass-2 stts on gpsimd; 2: all on gpsimd
TH = 34  # column split: the scans run as two ~40-col passes, pass 2
LAG = 2  # trailing pass 1 by LAG rows as an independent dependency chain

F32 = mybir.dt.float32
ALU = mybir.AluOpType
ACTF = mybir.ActivationFunctionType


def build_nc() -> bass.Bass:
    nc = bacc.Bacc("TRN2", target_bir_lowering=False)

    # cols [0:L) ini | [L:2L) msk | [2L:) scores, s-major, T1 per row
    sc_d = nc.dram_tensor("sc", [NP, SM + L * T1], F32, kind="ExternalInput")
    col_d = nc.dram_tensor("col", [NP, ROWS], F32, kind="ExternalOutput")

    with tile.TileContext(nc) as tc:
        with (
            tc.tile_pool(name="big", bufs=1) as bigp,
            tc.tile_pool(name="small", bufs=1) as smallp,
        ):
            SC = bigp.tile([NP, SM + L * T1], F32, tag="SC")
            P = bigp.tile([NP, L * T1], F32, tag="P")
            A = bigp.tile([NP, ROWS, COLS], F32, tag="A")
            U = bigp.tile([NP, NODD, T1], F32, tag="U")
            XC = smallp.tile([NP, ROWS], F32, tag="XC")

            # Pad zeroing on the (idle until the first scan) Vector engine.
            nc.vector.memset(A[:, 0:2, :], 0.0)
            nc.vector.memset(A[:, 2:ROWS, 0], 0.0)
            nc.vector.memset(U[:, :, 0], 0.0)

            # Score upload + exp, chunked over s so the row loop starts
            # after the first (small) chunk, which also carries ini|msk.
            # exp boundaries: prefix sums of EXPS, clipped to [0, L]
            ebnd = [0]
            for e in EXPS:
                if ebnd[-1] >= L:
                    break
                ebnd.append(min(L, ebnd[-1] + e))
            while ebnd[-1] < L:
                ebnd.append(min(L, ebnd[-1] + EXPS[-1]))

            lo = 0
            done_exp = 0
            for c, ch in enumerate(CHUNKS):
                hi = lo + ch
                st = 0 if c == 0 else SM + lo * T1
                nc.sync.dma_start(
                    out=SC[:, st : SM + hi * T1], in_=sc_d[:, st : SM + hi * T1]
                )
                # emit every exp op fully covered by the DMAs issued so far
                while done_exp + 1 < len(ebnd) and ebnd[done_exp + 1] <= hi:
                    elo, ehi = ebnd[done_exp], ebnd[done_exp + 1]
                    nc.scalar.activation(
                        P[:, elo * T1 : ehi * T1],
                        SC[:, SM + elo * T1 : SM + ehi * T1],
                        ACTF.Exp,
                    )
                    done_exp += 1
                lo = hi

            # Row loop: both directions fused per instruction; each row is a
            # stt + scan over cols [0, TH) (pass 1) and, LAG rows behind, the
            # same over [TH, T1) (pass 2, seeded by pass 1's last state) -
            # two interleaved dependency chains keep the engine busy through
            # the per-op semaphore/ack latency.
            def row_pass(s, p):
                r = s + 2
                clo, chi = (0, TH) if p == 0 else (TH, T1)
                if s >= 3 and s % 2 == 1:
                    j = (s - 3) // 2
                    eng = (
                        nc.gpsimd
                        if STT_POOL == 2 or (STT_POOL == 1 and p == 1)
                        else nc.vector
                    )
                    eng.scalar_tensor_tensor(
                        out=U[:, j, max(clo, 1) : chi],
                        in0=A[:, r - 2, max(clo, 1) : chi],
                        scalar=SC[:, L + s : L + s + 1],
                        in1=A[:, r - 1, max(clo, 1) : chi],
                        op0=ALU.mult, op1=ALU.add,
                    )
                    d_ap = U[:, j, clo:chi]
                else:
                    # u == alpha[s-1] shifted; pad column supplies u_0 = 0.
                    d_ap = A[:, r - 1, clo:chi]
                nc.vector.tensor_tensor_scan(
                    out=A[:, r, clo + 1 : chi + 1], data0=d_ap,
                    data1=P[:, s * T1 + clo : s * T1 + chi],
                    initial=SC[:, s : s + 1] if p == 0 else A[:, r, TH : TH + 1],
                    op0=ALU.add, op1=ALU.mult,
                )

            for s in range(L):
                row_pass(s, 0)
                if s >= LAG:
                    row_pass(s - LAG, 1)
            for s in range(L - LAG, L):
                row_pass(s, 1)

            # Boundary columns out (contiguous copy, then one plain DMA).
            nc.vector.tensor_copy(XC[:, :], A[:, 0:ROWS, T1])
            nc.sync.dma_start(out=col_d[:, :], in_=XC[:, :])

    nc.finalize()
    return nc


def host_prep(predictions, targets, target_lengths):
    """Shard + index prep: gather extended-target score rows, center them,
    and pack forward (partitions 0-7) and reversed-backward (8-15) halves."""
    predictions = np.asarray(predictions, dtype=np.float32)
    targets = np.asarray(targets)
    target_lengths = np.asarray(target_lengths)

    ext = np.zeros((B, L), dtype=np.int64)
    ext[:, 1::2] = targets
    m = np.zeros((B, L), dtype=np.float32)
    m[:, 3::2] = (targets[:, 1:] != targets[:, :-1]).astype(np.float32)
    accept = np.zeros((B, L), dtype=np.float32)
    idx = (2 * target_lengths).astype(np.int64)
    accept[np.arange(B), idx] = 1.0
    accept[np.arange(B), idx - 1] = 1.0

    # g[b, s, t] = predictions[t, b, ext[b, s]]
    g = predictions[:, np.arange(B)[:, None], ext]  # [T, B, L]
    g = np.ascontiguousarray(g.transpose(1, 2, 0))  # [B, L, T]
    # Per-(b,t) centering keeps the prob-domain DP inside fp32 range.  The
    # mean alone is not scale-adaptive (the forward drift per step is about
    # 0.85 sigma, the expected max over the ~2-3 reachable states), so add
    # 0.85*std: the drift then stays near zero for any score scale.
    logc = g.mean(axis=1) + np.float32(0.85) * g.std(axis=1)  # [B, T]
    gc = g - logc[:, None, :]

    # backward: rows r <-> s = 50-r, time reversed; mask mb[r] = m[52-r]
    gb = gc[:, ::-1, ::-1][:, :, : T - T1]
    mb = np.zeros((B, L), dtype=np.float32)
    mb[:, 2:] = m[:, 2:][:, ::-1]  # mb[r] = m[52-r], r in [2, 50]
    ib = accept[:, ::-1]  # ib[r] = accept[50-r]
    inif = np.zeros((B, L), dtype=np.float32)
    inif[:, 0:2] = 1.0

    in_maps = []
    for k in range(NCORES):
        bsl = slice(k * BLOC, (k + 1) * BLOC)
        sc = np.concatenate(
            [
                np.concatenate([inif[bsl], m[bsl]], axis=1),
                gc[bsl, :, :T1].reshape(BLOC, L * T1),
            ],
            axis=1,
        )
        sb = np.concatenate(
            [
                np.concatenate([ib[bsl], mb[bsl]], axis=1),
                gb[bsl].reshape(BLOC, L * T1),
            ],
            axis=1,
        )
        in_maps.append({"sc": np.ascontiguousarray(np.concatenate([sc, sb]))})
    return in_maps, (logc, mb)


_NC_CACHE = {}


def kernel(predictions, targets, target_lengths):
    if "nc" not in _NC_CACHE:
        _NC_CACHE["nc"] = build_nc()
    nc = _NC_CACHE["nc"]

    in_maps, aux = host_prep(predictions, targets, target_lengths)
    res = run_bass_kernel_spmd(nc, in_maps, core_ids=list(range(NCORES)))
    return finish(res.results, target_lengths, aux)


def finish(results, target_lengths, aux):
    logc, mb = aux
    cols = np.concatenate(
        [r["col"].reshape(NP, ROWS) for r in results]
    ).astype(np.float64)  # per core: rows 0:8 fwd col, 8:16 bwd col
    af = np.concatenate([cols[k * NP : k * NP + BLOC, 2:] for k in range(NCORES)])
    db = np.concatenate(
        [cols[k * NP + BLOC : (k + 1) * NP, :] for k in range(NCORES)]
    )
    # eta[r] = beta[r] + beta[r-1] + mb[r]*beta[r-2]; dot reverses r vs s
    eta = db[:, 2:] + db[:, 1:-1] + mb * db[:, :-2]
    tot = (af * eta[:, ::-1]).sum(axis=1)
    with np.errstate(divide="ignore", invalid="ignore"):
        nll = -(np.log(tot) + logc.astype(np.float64).sum(axis=1))
    lengths = np.asarray(target_lengths).astype(np.float64)
    per = np.where(~np.isfinite(nll) | (nll >= 1e29), 0.0, nll / lengths)
    return np.array(per.mean(), dtype=np.float32)


# revision 23
# speedup vs baseline: 1.3139x; 1.3139x over previous
"""CTC loss (nn.CTCLoss, blank=0, reduction='mean', zero_infinity=True) for
T=160, B=64, C=6625, S=25 on 8 TRN2 NeuronCores.

Sharding: data-parallel over batch - 8 of the 64 samples per core.

Algorithm: the CTC forward DP is lower-triangular in the extended-target
index s (alpha[s] depends only on alpha[s], alpha[s-1], alpha[s-2]), so it
is solved ROW-BY-ROW over s instead of step-by-step over t.  Row s is one
first-order linear recurrence along t,

    alpha_t[s] = (u_t + alpha_{t-1}[s]) * p_t[s],
    u_t        = alpha_{t-1}[s-1] + m[s] * alpha_{t-1}[s-2],

i.e. ONE hardware prefix-scan (tensor_tensor_scan, op0=add/op1=mult, fp32
state) over the whole time axis, preceded by one scalar_tensor_tensor for
the u row - and u degenerates to the raw alpha[s-1] row (pure AP shift into
the zero pad column) whenever m[s] is structurally zero: even s and s=1,
i.e. 27 of the 51 rows need no u op at all.

T is split in half: a forward chain over t<80 and a mirrored backward
(beta) chain over t>=80.  Because the scan is per-partition independent
and its cost depends only on free-axis length, BOTH chains ride in the
SAME instructions: partitions 0-7 carry the 8 forward samples, partitions
8-15 the backward ones (mask/init/score data selects the direction), so
the whole DP is 51 scans + 24 stts of [16, 80].  The device returns the
two boundary columns (alpha_79 / beta-hat_80, 53 values per sample); the
host contracts them (a 51-element masked dot per sample) while assembling
the final loss - the same finishing step that already applies the log and
the batch mean.

The module is built with RAW bass (no TileContext): semaphores guard only
the cross-engine edges (DMA chunk -> exp on the Activation engine -> the
row loop on the Vector engine, and boundary-column copy -> output DMA).
Same-engine read-after-write ordering relies on program order alone, which
the hardware path honours (validated empirically with a chain of
serially-dependent unsynced Vector ops); dropping the per-op semaphore
wait (~95ns of SBUF-write-ack + propagation per dependent hop) makes the
75-instruction row loop run back-to-back at pure processing rate.

Probabilities stay in fp32 with NO per-step rescaling: the host subtracts
a per-(b,t) centering constant (mean + 0.85*std of the extended-target
scores, scale-adaptive) before upload, which keeps all alpha/beta
magnitudes inside fp32 range; the host adds the centering log-sums back
when assembling the final loss.
"""

import numpy as np

import concourse.bacc as bacc
import concourse.bass as bass
import concourse.mybir as mybir
from concourse.bass_utils import run_bass_kernel_spmd

T = 160
B = 64
C = 6625
S = 25
L = 2 * S + 1  # 51
NCORES = 8
BLOC = B // NCORES  # 8 samples per core
T1 = 80  # forward half; backward covers T-T1 = 80
NP = 2 * BLOC  # 16 partitions: 0-7 forward, 8-15 backward
ROWS = L + 2  # 2 leading zero-pad rows (taps s-1, s-2 of rows 0/1)
COLS = T1 + 1  # leading zero-pad column (alpha_{-1})
NODD = 24  # odd rows s=3,5,...,49 need the u-op
SM = 2 * L  # leading ini|msk columns of the staged input
CHUNKS = [4, 8, 12, 27]  # s-rows per DMA chunk (chunk 0 also carries ini|msk)
EXPS = [2, 2, 8, 12, 27]  # s-rows per exp op (fine early, coarse late)

F32 = mybir.dt.float32
ALU = mybir.AluOpType
ACTF = mybir.ActivationFunctionType


def build_nc() -> bass.Bass:
    nc = bacc.Bacc("TRN2", target_bir_lowering=False)

    # cols [0:L) ini | [L:2L) msk | [2L:) scores, s-major, T1 per row
    sc_d = nc.dram_tensor("sc", [NP, SM + L * T1], F32, kind="ExternalInput")
    col_d = nc.dram_tensor("col", [NP, ROWS], F32, kind="ExternalOutput")

    SC = nc.alloc_sbuf_tensor("SC", [NP, SM + L * T1], F32)
    P = nc.alloc_sbuf_tensor("P", [NP, L * T1], F32)
    A = nc.alloc_sbuf_tensor("A", [NP, ROWS, COLS], F32)
    U = nc.alloc_sbuf_tensor("U", [NP, NODD, T1], F32)
    XC = nc.alloc_sbuf_tensor("XC", [NP, ROWS], F32)

    sem_d = nc.alloc_semaphore("sd")  # DMA chunk arrivals (+16 each)
    sem_e = nc.alloc_semaphore("se")  # exp completions (+1 each)
    sem_o = nc.alloc_semaphore("so")  # copy -> out-DMA -> end

    # exp boundaries: prefix sums of EXPS, clipped to [0, L]
    ebnd = [0]
    for e in EXPS:
        if ebnd[-1] >= L:
            break
        ebnd.append(min(L, ebnd[-1] + e))
    while ebnd[-1] < L:
        ebnd.append(min(L, ebnd[-1] + EXPS[-1]))

    # Score upload on the SP queue; each exp waits its chunk's semaphore.
    lo = 0
    done_exp = 0
    for ci, ch in enumerate(CHUNKS):
        hi = lo + ch
        st = 0 if ci == 0 else SM + lo * T1
        nc.sync.dma_start(
            out=SC[:, st : SM + hi * T1], in_=sc_d[:, st : SM + hi * T1]
        ).then_inc(sem_d, 16)
        while done_exp + 1 < len(ebnd) and ebnd[done_exp + 1] <= hi:
            elo, ehi = ebnd[done_exp], ebnd[done_exp + 1]
            nc.scalar.wait_ge(sem_d, 16 * (ci + 1))
            nc.scalar.activation(
                P[:, elo * T1 : ehi * T1],
                SC[:, SM + elo * T1 : SM + ehi * T1],
                ACTF.Exp,
            ).then_inc(sem_e, 1)
            done_exp += 1
        lo = hi

    # Pad zeroing on the Vector engine (program order before the first scan).
    nc.vector.memset(A[:, 0:2, :], 0.0)
    nc.vector.memset(A[:, 2:ROWS, 0], 0.0)
    nc.vector.memset(U[:, :, 0], 0.0)

    # Row loop: full-width, both directions fused per instruction; the only
    # waits are on the exp chunks - consecutive Vector ops chain through
    # program order with no semaphores.
    need_exp = 0
    for s in range(L):
        r = s + 2
        while ebnd[need_exp + 1] <= s:
            need_exp += 1
        nc.vector.wait_ge(sem_e, need_exp + 1)
        if s >= 3 and s % 2 == 1:
            j = (s - 3) // 2
            nc.vector.scalar_tensor_tensor(
                out=U[:, j, 1:T1], in0=A[:, r - 2, 1:T1],
                scalar=SC[:, L + s : L + s + 1], in1=A[:, r - 1, 1:T1],
                op0=ALU.mult, op1=ALU.add,
            )
            d_ap = U[:, j, 0:T1]
        else:
            # u == alpha[s-1] shifted; the pad column supplies u_0 = 0.
            d_ap = A[:, r - 1, 0:T1]
        nc.vector.tensor_tensor_scan(
            out=A[:, r, 1:COLS], data0=d_ap,
            data1=P[:, s * T1 : (s + 1) * T1],
            initial=SC[:, s : s + 1],
            op0=ALU.add, op1=ALU.mult,
        )

    # Boundary columns out (contiguous copy, then one plain DMA).
    nc.vector.tensor_copy(XC[:, :], A[:, 0:ROWS, T1]).then_inc(sem_o, 1)
    nc.sync.wait_ge(sem_o, 1)
    nc.sync.dma_start(out=col_d[:, :], in_=XC[:, :]).then_inc(sem_o, 16)
    nc.sync.wait_ge(sem_o, 17)

    nc.finalize()
    return nc


def host_prep(predictions, targets, target_lengths):
    """Shard + index prep: gather extended-target score rows, center them,
    and pack forward (partitions 0-7) and reversed-backward (8-15) halves."""
    predictions = np.asarray(predictions, dtype=np.float32)
    targets = np.asarray(targets)
    target_lengths = np.asarray(target_lengths)

    ext = np.zeros((B, L), dtype=np.int64)
    ext[:, 1::2] = targets
    m = np.zeros((B, L), dtype=np.float32)
    m[:, 3::2] = (targets[:, 1:] != targets[:, :-1]).astype(np.float32)
    accept = np.zeros((B, L), dtype=np.float32)
    idx = (2 * target_lengths).astype(np.int64)
    accept[np.arange(B), idx] = 1.0
    accept[np.arange(B), idx - 1] = 1.0

    # g[b, s, t] = predictions[t, b, ext[b, s]]
    g = predictions[:, np.arange(B)[:, None], ext]  # [T, B, L]
    g = np.ascontiguousarray(g.transpose(1, 2, 0))  # [B, L, T]
    # Per-(b,t) centering keeps the prob-domain DP inside fp32 range.  The
    # mean alone is not scale-adaptive (the forward drift per step is about
    # 0.85 sigma, the expected max over the ~2-3 reachable states), so add
    # 0.85*std: the drift then stays near zero for any score scale.
    logc = g.mean(axis=1) + np.float32(0.85) * g.std(axis=1)  # [B, T]
    gc = g - logc[:, None, :]

    # backward: rows r <-> s = 50-r, time reversed; mask mb[r] = m[52-r]
    gb = gc[:, ::-1, ::-1][:, :, : T - T1]
    mb = np.zeros((B, L), dtype=np.float32)
    mb[:, 2:] = m[:, 2:][:, ::-1]  # mb[r] = m[52-r], r in [2, 50]
    ib = accept[:, ::-1]  # ib[r] = accept[50-r]
    inif = np.zeros((B, L), dtype=np.float32)
    inif[:, 0:2] = 1.0

    in_maps = []
    for k in range(NCORES):
        bsl = slice(k * BLOC, (k + 1) * BLOC)
        sc = np.concatenate(
            [
                np.concatenate([inif[bsl], m[bsl]], axis=1),
                gc[bsl, :, :T1].reshape(BLOC, L * T1),
            ],
            axis=1,
        )
        sb = np.concatenate(
            [
                np.concatenate([ib[bsl], mb[bsl]], axis=1),
                gb[bsl].reshape(BLOC, L * T1),
            ],
            axis=1,
        )
        in_maps.append({"sc": np.ascontiguousarray(np.concatenate([sc, sb]))})
    return in_maps, (logc, mb)


_NC_CACHE = {}


def kernel(predictions, targets, target_lengths):
    if "nc" not in _NC_CACHE:
        _NC_CACHE["nc"] = build_nc()
    nc = _NC_CACHE["nc"]

    in_maps, aux = host_prep(predictions, targets, target_lengths)
    res = run_bass_kernel_spmd(nc, in_maps, core_ids=list(range(NCORES)))
    return finish(res.results, target_lengths, aux)


def finish(results, target_lengths, aux):
    logc, mb = aux
    cols = np.concatenate(
        [r["col"].reshape(NP, ROWS) for r in results]
    ).astype(np.float64)  # per core: rows 0:8 fwd col, 8:16 bwd col
    af = np.concatenate([cols[k * NP : k * NP + BLOC, 2:] for k in range(NCORES)])
    db = np.concatenate(
        [cols[k * NP + BLOC : (k + 1) * NP, :] for k in range(NCORES)]
    )
    # eta[r] = beta[r] + beta[r-1] + mb[r]*beta[r-2]; dot reverses r vs s
    eta = db[:, 2:] + db[:, 1:-1] + mb * db[:, :-2]
    tot = (af * eta[:, ::-1]).sum(axis=1)
    with np.errstate(divide="ignore", invalid="ignore"):
        nll = -(np.log(tot) + logc.astype(np.float64).sum(axis=1))
    lengths = np.asarray(target_lengths).astype(np.float64)
    per = np.where(~np.isfinite(nll) | (nll >= 1e29), 0.0, nll / lengths)
    return np.array(per.mean(), dtype=np.float32)


# revision 24
# speedup vs baseline: 1.3390x; 1.0192x over previous
"""CTC loss (nn.CTCLoss, blank=0, reduction='mean', zero_infinity=True) for
T=160, B=64, C=6625, S=25 on 8 TRN2 NeuronCores.

Sharding: data-parallel over batch - 8 of the 64 samples per core.

Algorithm: the CTC forward DP is lower-triangular in the extended-target
index s (alpha[s] depends only on alpha[s], alpha[s-1], alpha[s-2]), so it
is solved ROW-BY-ROW over s instead of step-by-step over t.  Row s is one
first-order linear recurrence along t,

    alpha_t[s] = (u_t + alpha_{t-1}[s]) * p_t[s],
    u_t        = alpha_{t-1}[s-1] + m[s] * alpha_{t-1}[s-2],

i.e. ONE hardware prefix-scan (tensor_tensor_scan, op0=add/op1=mult, fp32
state) over the whole time axis, preceded by one scalar_tensor_tensor for
the u row - and u degenerates to the raw alpha[s-1] row (pure AP shift into
the zero pad column) whenever m[s] is structurally zero: even s and s=1,
i.e. 27 of the 51 rows need no u op at all.

T is split in half: a forward chain over t<80 and a mirrored backward
(beta) chain over t>=80.  Because the scan is per-partition independent
and its cost depends only on free-axis length, BOTH chains ride in the
SAME instructions: partitions 0-7 carry the 8 forward samples, partitions
8-15 the backward ones (mask/init/score data selects the direction), so
the whole DP is 51 scans + 24 stts of [16, 80].  The device returns the
two boundary columns (alpha_79 / beta-hat_80, 53 values per sample); the
host contracts them (a 51-element masked dot per sample) while assembling
the final loss - the same finishing step that already applies the log and
the batch mean.

The module is built with RAW bass (no TileContext): semaphores guard only
the cross-engine edges (DMA chunk -> exp on the Activation engine -> the
row loop on the Vector engine, and boundary-column copy -> output DMA).
Same-engine read-after-write ordering relies on program order alone, which
the hardware path honours (validated empirically with a chain of
serially-dependent unsynced Vector ops); dropping the per-op semaphore
wait (~95ns of SBUF-write-ack + propagation per dependent hop) makes the
75-instruction row loop run back-to-back at pure processing rate.

Probabilities stay in fp32 with NO per-step rescaling: the host subtracts
a per-(b,t) centering constant (mean + 0.85*std of the extended-target
scores, scale-adaptive) before upload, which keeps all alpha/beta
magnitudes inside fp32 range; the host adds the centering log-sums back
when assembling the final loss.
"""

import numpy as np

import concourse.bacc as bacc
import concourse.bass as bass
import concourse.mybir as mybir
from concourse.bass_utils import run_bass_kernel_spmd

T = 160
B = 64
C = 6625
S = 25
L = 2 * S + 1  # 51
NCORES = 8
BLOC = B // NCORES  # 8 samples per core
T1 = 80  # forward half; backward covers T-T1 = 80
NP = 2 * BLOC  # 16 partitions: 0-7 forward, 8-15 backward
ROWS = L + 2  # 2 leading zero-pad rows (taps s-1, s-2 of rows 0/1)
COLS = T1 + 1  # leading zero-pad column (alpha_{-1})
NODD = 24  # odd rows s=3,5,...,49 need the u-op
SM = 2 * L  # leading ini|msk columns of the staged input
CHUNKS = [4, 4, 6, 10, 27]  # s-rows per DMA chunk (chunk 0 also carries ini|msk)
EXPS = [2, 2, 4, 6, 10, 27]  # s-rows per exp op (fine early, coarse late)

F32 = mybir.dt.float32
ALU = mybir.AluOpType
ACTF = mybir.ActivationFunctionType


def build_nc() -> bass.Bass:
    nc = bacc.Bacc("TRN2", target_bir_lowering=False)

    # cols [0:L) ini | [L:2L) msk | [2L:) scores, s-major, T1 per row
    sc_d = nc.dram_tensor("sc", [NP, SM + L * T1], F32, kind="ExternalInput")
    col_d = nc.dram_tensor("col", [NP, ROWS], F32, kind="ExternalOutput")

    SC = nc.alloc_sbuf_tensor("SC", [NP, SM + L * T1], F32)
    P = nc.alloc_sbuf_tensor("P", [NP, L * T1], F32)
    A = nc.alloc_sbuf_tensor("A", [NP, ROWS, COLS], F32)
    U = nc.alloc_sbuf_tensor("U", [NP, NODD, T1], F32)
    XC = nc.alloc_sbuf_tensor("XC", [NP, ROWS], F32)

    sem_d = nc.alloc_semaphore("sd")  # DMA chunk arrivals (+16 each)
    sem_e = nc.alloc_semaphore("se")  # exp completions (+1 each)
    sem_o = nc.alloc_semaphore("so")  # copy -> out-DMA -> end

    # exp boundaries: prefix sums of EXPS, clipped to [0, L]
    ebnd = [0]
    for e in EXPS:
        if ebnd[-1] >= L:
            break
        ebnd.append(min(L, ebnd[-1] + e))
    while ebnd[-1] < L:
        ebnd.append(min(L, ebnd[-1] + EXPS[-1]))

    # Score upload on the SP queue; each exp waits its chunk's semaphore.
    lo = 0
    done_exp = 0
    for ci, ch in enumerate(CHUNKS):
        hi = lo + ch
        st = 0 if ci == 0 else SM + lo * T1
        nc.sync.dma_start(
            out=SC[:, st : SM + hi * T1], in_=sc_d[:, st : SM + hi * T1]
        ).then_inc(sem_d, 16)
        while done_exp + 1 < len(ebnd) and ebnd[done_exp + 1] <= hi:
            elo, ehi = ebnd[done_exp], ebnd[done_exp + 1]
            nc.scalar.wait_ge(sem_d, 16 * (ci + 1))
            nc.scalar.activation(
                P[:, elo * T1 : ehi * T1],
                SC[:, SM + elo * T1 : SM + ehi * T1],
                ACTF.Exp,
            ).then_inc(sem_e, 1)
            done_exp += 1
        lo = hi

    # Pad zeroing on the Vector engine (program order before the first scan).
    nc.vector.memset(A[:, 0:2, :], 0.0)
    nc.vector.memset(A[:, 2:ROWS, 0], 0.0)
    nc.vector.memset(U[:, :, 0], 0.0)

    # Row loop: full-width, both directions fused per instruction; the only
    # waits are on the exp chunks - consecutive Vector ops chain through
    # program order with no semaphores.
    need_exp = 0
    for s in range(L):
        r = s + 2
        while ebnd[need_exp + 1] <= s:
            need_exp += 1
        nc.vector.wait_ge(sem_e, need_exp + 1)
        if s >= 3 and s % 2 == 1:
            j = (s - 3) // 2
            nc.vector.scalar_tensor_tensor(
                out=U[:, j, 1:T1], in0=A[:, r - 2, 1:T1],
                scalar=SC[:, L + s : L + s + 1], in1=A[:, r - 1, 1:T1],
                op0=ALU.mult, op1=ALU.add,
            )
            d_ap = U[:, j, 0:T1]
        else:
            # u == alpha[s-1] shifted; the pad column supplies u_0 = 0.
            d_ap = A[:, r - 1, 0:T1]
        nc.vector.tensor_tensor_scan(
            out=A[:, r, 1:COLS], data0=d_ap,
            data1=P[:, s * T1 : (s + 1) * T1],
            initial=SC[:, s : s + 1],
            op0=ALU.add, op1=ALU.mult,
        )

    # Boundary columns out (contiguous copy, then one plain DMA).
    nc.vector.tensor_copy(XC[:, :], A[:, 0:ROWS, T1]).then_inc(sem_o, 1)
    nc.sync.wait_ge(sem_o, 1)
    nc.sync.dma_start(out=col_d[:, :], in_=XC[:, :]).then_inc(sem_o, 16)
    nc.sync.wait_ge(sem_o, 17)

    nc.finalize()
    return nc


def host_prep(predictions, targets, target_lengths):
    """Shard + index prep: gather extended-target score rows, center them,
    and pack forward (partitions 0-7) and reversed-backward (8-15) halves."""
    predictions = np.asarray(predictions, dtype=np.float32)
    targets = np.asarray(targets)
    target_lengths = np.asarray(target_lengths)

    ext = np.zeros((B, L), dtype=np.int64)
    ext[:, 1::2] = targets
    m = np.zeros((B, L), dtype=np.float32)
    m[:, 3::2] = (targets[:, 1:] != targets[:, :-1]).astype(np.float32)
    accept = np.zeros((B, L), dtype=np.float32)
    idx = (2 * target_lengths).astype(np.int64)
    accept[np.arange(B), idx] = 1.0
    accept[np.arange(B), idx - 1] = 1.0

    # g[b, s, t] = predictions[t, b, ext[b, s]]
    g = predictions[:, np.arange(B)[:, None], ext]  # [T, B, L]
    g = np.ascontiguousarray(g.transpose(1, 2, 0))  # [B, L, T]
    # Per-(b,t) centering keeps the prob-domain DP inside fp32 range.  The
    # mean alone is not scale-adaptive (the forward drift per step is about
    # 0.85 sigma, the expected max over the ~2-3 reachable states), so add
    # 0.85*std: the drift then stays near zero for any score scale.
    logc = g.mean(axis=1) + np.float32(0.85) * g.std(axis=1)  # [B, T]
    gc = g - logc[:, None, :]

    # backward: rows r <-> s = 50-r, time reversed; mask mb[r] = m[52-r]
    gb = gc[:, ::-1, ::-1][:, :, : T - T1]
    mb = np.zeros((B, L), dtype=np.float32)
    mb[:, 2:] = m[:, 2:][:, ::-1]  # mb[r] = m[52-r], r in [2, 50]
    ib = accept[:, ::-1]  # ib[r] = accept[50-r]
    inif = np.zeros((B, L), dtype=np.float32)
    inif[:, 0:2] = 1.0

    in_maps = []
    for k in range(NCORES):
        bsl = slice(k * BLOC, (k + 1) * BLOC)
        sc = np.concatenate(
            [
                np.concatenate([inif[bsl], m[bsl]], axis=1),
                gc[bsl, :, :T1].reshape(BLOC, L * T1),
            ],
            axis=1,
        )
        sb = np.concatenate(
            [
                np.concatenate([ib[bsl], mb[bsl]], axis=1),
                gb[bsl].reshape(BLOC, L * T1),
            ],
            axis=1,
        )
        in_maps.append({"sc": np.ascontiguousarray(np.concatenate([sc, sb]))})
    return in_maps, (logc, mb)


_NC_CACHE = {}


def kernel(predictions, targets, target_lengths):
    if "nc" not in _NC_CACHE:
        _NC_CACHE["nc"] = build_nc()
    nc = _NC_CACHE["nc"]

    in_maps, aux = host_prep(predictions, targets, target_lengths)
    res = run_bass_kernel_spmd(nc, in_maps, core_ids=list(range(NCORES)))
    return finish(res.results, target_lengths, aux)


def finish(results, target_lengths, aux):
    logc, mb = aux
    cols = np.concatenate(
        [r["col"].reshape(NP, ROWS) for r in results]
    ).astype(np.float64)  # per core: rows 0:8 fwd col, 8:16 bwd col
    af = np.concatenate([cols[k * NP : k * NP + BLOC, 2:] for k in range(NCORES)])
    db = np.concatenate(
        [cols[k * NP + BLOC : (k + 1) * NP, :] for k in range(NCORES)]
    )
    # eta[r] = beta[r] + beta[r-1] + mb[r]*beta[r-2]; dot reverses r vs s
    eta = db[:, 2:] + db[:, 1:-1] + mb * db[:, :-2]
    tot = (af * eta[:, ::-1]).sum(axis=1)
    with np.errstate(divide="ignore", invalid="ignore"):
        nll = -(np.log(tot) + logc.astype(np.float64).sum(axis=1))
    lengths = np.asarray(target_lengths).astype(np.float64)
    per = np.where(~np.isfinite(nll) | (nll >= 1e29), 0.0, nll / lengths)
    return np.array(per.mean(), dtype=np.float32)
